# revision 47
# baseline (speedup 1.0000x reference)
"""MoE v6.12: expert-parallel, mixed-precision bf16/fp8 work units.

Each (token, expert) pair is classed by its gate weight: low-weight
pairs run in fp8-e4m3 DoubleRow matmuls (exactly 2x bf16 column rate
on HW: N/2.4GHz per 2 ko-tiles), the high-weight rest stay bf16.  The
cutoff is chosen at run time from the realized routing stats
(setup_inputs is not bit-identical across jax backends) to cap
predicted rel err at ~1.8e-2 against the 2e-2 budget (measured err
tracks 0.975*TARGET_ERR; validated 1.805e-2 on HW).

Work unit = (expert e, output quarter q, class) costing the class's
token count for e.  Per class, the 32 units are sorted by cost and
dealt into 4 uniform slots of 8 (one unit per core per slot); slot
capacity = max count in the slot.  Slots execute smallest-first
(ASC) so the head DMA fill is minimal; slot sequence interleaves
classes F0 B0 F1 B1 F2 B2 F3 B3 (F = fp8, B = bf16).  The last bf16
slot's ctile list is recut to end on a 232-wide tile so the final
store drain is short.

v6.x changes vs v6 (196315ns baseline -> ~178000ns):
- Per-ctile DRAM tensors for x and y AND exact-shape SBUF tiles:
  every DMA is fully contiguous on both sides (128 descriptors of
  multi-KB lines instead of ~2k thin runs per tile).  Total
  descriptor count dropped ~5x; the DMA engines are the co-binding
  resource, so this is the main win (also keeps the PE out of the
  P0 power downclock observed at ~2.0GHz).
- TARGET_ERR 0.017 -> 0.0185 (more fp8 fraction).
- Slots ordered smallest-first; 10 warm-up matmuls cover the head
  DMA fill so real matmuls start at the warm 2.4GHz clock.
- Per-expert class-boundary optimization (hill-climb): shifting
  boundary tokens between classes per expert zeroes most of the deal
  padding in BOTH classes and buys extra fp8 fraction, under a hard
  predicted-error cap of 0.019 (measured err = 0.975*pred = 1.853e-2,
  7.3%% margin).  xf0c1 rides the sync ring so the two DMA rings
  split the head fill evenly.
- ko-outer head (v6.12): slot f0's weights are packed ko-chunk-major
  ([4, P, OTS, 4, P], four 256KB chunk tiles) and its first ctile
  accumulates o=0..3 in four parallel PSUM banks while consuming
  W/x ko-chunks as they land — the first real matmuls need only
  ~500KB of head DMA instead of ~1.9MB.  Measured 172304ns at full
  clock (vs 173537 without).

fp8 scaling: x*16, W*128 quantized to e4m3 (TRN variant, max 240);
bias pre-scaled by 2048 on host, combine weight divided by 2048 on
host, using relu(s*z) = s*relu(z).

DMA: two HWDGE queues.  sync = xF0c0 head + all bf16 xt + all stores;
scalar = all weights/biases + remaining fp8 xt.  Emission per slot
position keeps each engine's FIFO in deadline order so stores never
sit behind loads that block on pool-buffer recycling.  Keep x/W tile
loads as several dma_starts (not one monolithic dma): a single ring
serves a dma's descriptors with limited engine parallelism, so
monolithic loads lengthen the critical head fill (measured +6us).
"""

import numpy as np
import ml_dtypes

N, D, E, TOP_K = 8192, 2048, 8, 2
P = 128
KO = D // P          # 16 contraction tiles
NSLOTS = 4           # per class
OTS = 4              # o-chunks per quarter (4 x 128 = 512 out channels)
QW = D // NSLOTS     # 512

# Adaptive fp8 threshold: the realized routing stats differ by jax
# backend (setup_inputs is not bit-identical across platforms), so the
# gate-weight cutoff is chosen at run time to cap the predicted rel
# err.  err^2 ~= K_ERR * (sum of w^2 below cutoff / total w^2) + BASE^2,
# with K_ERR calibrated conservatively on measured realizations.
K_ERR = 1.18e-3
BASE_ERR = 0.0024
TARGET_ERR = 0.0185
SX, SW = 16.0, 128.0
SCOMB = SX * SW      # 2048

PROFILE = False
LAST_RESULTS = None

_KERNEL_CACHE = {}

F8 = ml_dtypes.float8_e4m3   # TRN fp8_e4m3: bias 7, max 240
BF16 = ml_dtypes.bfloat16


def _routing(x, W_gate, b_gate):
    import jax

    cpu = jax.devices("cpu")[0]
    with jax.default_device(cpu):
        xj = jax.device_put(np.asarray(x, dtype=np.float32), cpu)
        wg = jax.device_put(np.asarray(W_gate, dtype=np.float32), cpu)
        bg = jax.device_put(np.asarray(b_gate, dtype=np.float32), cpu)
        logits = xj @ wg.T + bg
        gate = jax.nn.softmax(logits, axis=-1)
        vals, idx = jax.lax.top_k(gate, TOP_K)
        vals, idx = np.asarray(vals), np.asarray(idx)
    return vals, idx


def _ctiles(C, g):
    # widths multiple of g, each <=512 and kept >=~340 where possible
    def split(rem):
        if rem <= 512:
            return [rem] if rem else []
        if rem <= 1024:
            hi = -(-(-(-rem // 2)) // g) * g
            return [hi, rem - hi]
        if rem <= 1536:
            t1 = -(-rem // (3 * g)) * g
            r2 = rem - t1
            t2 = -(-r2 // (2 * g)) * g
            return [t1, t2, r2 - t2]
        return [512] + split(rem - 512)

    widths = split(C)
    tiles, c0 = [], 0
    for w in widths:
        tiles.append((c0, w))
        c0 += w
    return tuple(tiles)


def _ctl_lists(capsb, capsf):
    ctlb = [_ctiles(C, 4) for C in capsb]
    ctlf = [_ctiles(C, 16) for C in capsf]
    # Last slot in SEQ is b[NSLOTS-1]: recut so its final ctile is small
    # (232 wide) to minimize the tail store drain; widths stay >=232 so
    # bf16 LDWEIGHTS (~96ns) remains hidden under the matmul.
    lc = capsb[NSLOTS - 1]
    if lc > 480:
        tail_w = 232
        rest = lc - tail_w
        if rest <= 512:
            ctlb[NSLOTS - 1] = ((0, rest), (rest, tail_w))
        else:
            h1 = -(-rest // 2 // 4) * 4
            ctlb[NSLOTS - 1] = ((0, h1), (h1, rest - h1), (rest, tail_w))
    return ctlb, ctlf


def _build(capsb, capsf):
    import concourse.tile as tile
    from concourse import bacc, mybir

    nc = bacc.Bacc("TRN2", target_bir_lowering=False, debug=False)

    ctlb, ctlf = _ctl_lists(capsb, capsf)

    # Per-ctile DRAM tensors: fully contiguous loads/stores.
    wtb, bb, ytb = [], [], []
    wtf, bf_, ytf = [], [], []
    xtb = {}   # (s, ci) -> ap
    xtf = {}
    for s, C in enumerate(capsb):
        wtb.append(nc.dram_tensor(
            f"wtb{s}", [OTS, P, KO, P], mybir.dt.bfloat16, kind="ExternalInput").ap())
        bb.append(nc.dram_tensor(
            f"bb{s}", [P, OTS], mybir.dt.float32, kind="ExternalInput").ap())
        ytb.append([nc.dram_tensor(
            f"ytb{s}_{ci}", [P, OTS, cw], mybir.dt.bfloat16,
            kind="ExternalOutput").ap() for ci, (_, cw) in enumerate(ctlb[s])])
        for ci, (_, cw) in enumerate(ctlb[s]):
            xtb[(s, ci)] = nc.dram_tensor(
                f"xtb{s}_{ci}", [P, KO, cw], mybir.dt.bfloat16,
                kind="ExternalInput").ap()
    for s, C in enumerate(capsf):
        shape = ([4, P, OTS, 4, P] if s == 0 else [OTS, P, KO, P])
        wtf.append(nc.dram_tensor(
            f"wtf{s}", shape, mybir.dt.float8e4, kind="ExternalInput").ap())
        bf_.append(nc.dram_tensor(
            f"bf{s}", [P, OTS], mybir.dt.float32, kind="ExternalInput").ap())
        ytf.append([nc.dram_tensor(
            f"ytf{s}_{ci}", [P, OTS, cw], mybir.dt.bfloat16,
            kind="ExternalOutput").ap() for ci, (_, cw) in enumerate(ctlf[s])])
        for ci, (_, cw) in enumerate(ctlf[s]):
            xtf[(s, ci)] = nc.dram_tensor(
                f"xtf{s}_{ci}", [P, KO, cw], mybir.dt.float8e4,
                kind="ExternalInput").ap()

    # slot sequence: (class, slot_idx); F first (small fp8 head fill)
    SEQ = [("f", 0), ("b", 0), ("f", 1), ("b", 1),
           ("f", 2), ("b", 2), ("f", 3), ("b", 3)]

    with tile.TileContext(nc) as tc:
        with (
            tc.tile_pool(name="consts", bufs=1) as cpool,
            tc.tile_pool(name="xb", bufs=3) as xbpool,
            tc.tile_pool(name="xf", bufs=4) as xfpool,
            tc.tile_pool(name="outs", bufs=6) as opool,
            tc.tile_pool(name="psum", bufs=8, space="PSUM") as pspool,
        ):
            # PE clock warm-up: HAM holds 1.2GHz until ~3.4us sustained
            # activity; burn part of the DMA head fill on scratch matmuls
            # (the first real matmuls continue the warm-up on useful work).
            warm = cpool.tile([P, 640], mybir.dt.bfloat16, name="warm")
            nc.vector.memset(warm[:], 0.0)
            wps = pspool.tile([P, 512], mybir.dt.float32, tag="ps")
            for _ in range(10):
                nc.tensor.matmul(wps[:], warm[:, :P], warm[:, :512],
                                 start=True, stop=True)
            nc.vector.tensor_copy(warm[:1, 512:516], wps[:1, :4])

            wtb_sb = [cpool.tile([P, OTS, KO, P], mybir.dt.bfloat16, name=f"wb{s}")
                      for s in range(NSLOTS)]
            wtf_sb = [None if s == 0 else
                      cpool.tile([P, OTS, KO, P], mybir.dt.float8e4, name=f"wf{s}")
                      for s in range(NSLOTS)]
            wf0c_sb = [cpool.tile([P, OTS, 4, P], mybir.dt.float8e4,
                                  name=f"wf0c{j}") for j in range(4)]
            bb_sb = [cpool.tile([P, OTS], mybir.dt.float32, name=f"bbs{s}")
                     for s in range(NSLOTS)]
            bf_sb = [cpool.tile([P, OTS], mybir.dt.float32, name=f"bfs{s}")
                     for s in range(NSLOTS)]
            xt_sb = {}  # (cls, s, ci) -> tile

            def load_x(cls, s, ci, ksplit=1):
                pool, xt, ctl = (
                    (xbpool, xtb, ctlb) if cls == "b"
                    else (xfpool, xtf, ctlf))
                dt = mybir.dt.bfloat16 if cls == "b" else mybir.dt.float8e4
                c0, cw = ctl[s][ci]
                t = pool.tile([P, KO, cw], dt, tag=f"x{cls}")
                xt_sb[(cls, s, ci)] = t
                eng = nc.sync if cls == "b" else nc.scalar
                if cls == "f" and s == 0 and ci <= 1:
                    eng = nc.sync   # balance the two rings on the head fill
                kc = KO // ksplit
                for i in range(ksplit):
                    eng.dma_start(
                        t[:, i * kc:(i + 1) * kc, :],
                        xt[(s, ci)][:, i * kc:(i + 1) * kc, :])

            def load_w(cls, s):
                wt, wt_sb_, bias, bias_sb_ = (
                    (wtb, wtb_sb, bb, bb_sb) if cls == "b"
                    else (wtf, wtf_sb, bf_, bf_sb))
                for o in range(OTS):
                    nc.scalar.dma_start(wt_sb_[s][:, o], wt[s][o])
                nc.scalar.dma_start(bias_sb_[s][:], bias[s][:])

            # ---- head loads ----
            load_x("f", 0, 0, ksplit=4)   # sync, ko-chunked
            if len(ctlf[0]) > 1:
                load_x("f", 0, 1)         # sync
            for j in range(4):
                nc.scalar.dma_start(wf0c_sb[j][:], wtf[0][j])
            nc.scalar.dma_start(bf_sb[0][:], bf_[0][:])
            for ci in range(2, len(ctlf[0])):
                load_x("f", 0, ci)        # scalar
            load_w("b", 0)

            # per-position prefetch emissions (order matters per engine):
            # scalar gets weights + fp8 xt, sync gets bf16 xt; each
            # engine's FIFO stays in deadline order so stores emitted in
            # the compute loop never sit behind far-future loads.
            def rng(ctl, s, a, b=None):
                n = len(ctl[s])
                return tuple(range(a, n if b is None else min(b, n)))

            prefetch = {p: [] for p in range(8)}
            for s in range(1, NSLOTS):
                prefetch[2 * (s - 1)] += [("wf", s), ("xf", s, (0,))]
                prefetch[2 * (s - 1) + 1] += [("xf", s, rng(ctlf, s, 1))]
                prefetch[2 * s] += [("wb", s)]
            prefetch[0] += [("xb", 0, rng(ctlb, 0, 0))]
            prefetch[1] += [("xb", 1, rng(ctlb, 1, 0))]
            prefetch[2] += [("xb", 2, (0,))]
            prefetch[3] += [("xb", 2, rng(ctlb, 2, 1))]
            prefetch[5] += [("xb", 3, rng(ctlb, 3, 0))]

            gi = 0  # global o-chunk counter for psum rotation
            for p, (cls, s) in enumerate(SEQ):
                for item in prefetch[p]:
                    kind = item[0]
                    if kind in ("wb", "wf"):
                        load_w(kind[1], item[1])
                    else:
                        c2, s2, cis = kind[1], item[1], item[2]
                        for ci in cis:
                            ctl2 = ctlb if c2 == "b" else ctlf
                            if ci < len(ctl2[s2]):
                                load_x(c2, s2, ci, ksplit=2 if (
                                    c2 == "b" and s2 == 0 and ci == 0) else 1)

                ctl = ctlb[s] if cls == "b" else ctlf[s]
                yt = ytb[s] if cls == "b" else ytf[s]
                wsb = wtb_sb[s] if cls == "b" else wtf_sb[s]
                bsb = bb_sb[s] if cls == "b" else bf_sb[s]
                for ci, (c0, cw) in enumerate(ctl):
                    xtile = xt_sb[(cls, s, ci)]
                    ot = opool.tile([P, OTS, cw], mybir.dt.bfloat16, tag="ot")
                    if cls == "f" and s == 0 and ci == 0:
                        pss = [pspool.tile([P, 512], mybir.dt.float32,
                                           tag="ps", name=f"psh{o_}")
                               for o_ in range(OTS)]
                        gi += OTS
                        for j in range(4):
                            for o in range(OTS):
                                for m in range(2):
                                    nc.tensor.matmul(
                                        pss[o][:, :cw],
                                        wf0c_sb[j][:, o, 2 * m:2 * m + 2],
                                        xtile[:, 4 * j + 2 * m:
                                              4 * j + 2 * m + 2, :cw],
                                        start=(j == 0 and m == 0),
                                        stop=(j == 3 and m == 1),
                                        perf_mode=(
                                            mybir.MatmulPerfMode.DoubleRow))
                        for o in range(OTS):
                            nc.vector.tensor_scalar(
                                ot[:, o, :cw], pss[o][:, :cw],
                                bsb[:, o:o + 1], 0.0,
                                mybir.AluOpType.add, mybir.AluOpType.max)
                        nc.sync.dma_start(yt[ci][:, :, :], ot[:, :, :cw])
                        continue
                    for o in range(OTS):
                        ps = pspool.tile([P, 512], mybir.dt.float32, tag="ps")
                        gi += 1
                        if cls == "b":
                            for ko in range(KO):
                                nc.tensor.matmul(
                                    ps[:, :cw],
                                    wsb[:, o, ko],
                                    xtile[:, ko, :cw],
                                    start=(ko == 0),
                                    stop=(ko == KO - 1))
                        elif s == 0:
                            for k in range(KO // 2):
                                nc.tensor.matmul(
                                    ps[:, :cw],
                                    wf0c_sb[k // 2][:, o,
                                                    2 * (k % 2):
                                                    2 * (k % 2) + 2],
                                    xtile[:, 2 * k:2 * k + 2, :cw],
                                    start=(k == 0),
                                    stop=(k == KO // 2 - 1),
                                    perf_mode=mybir.MatmulPerfMode.DoubleRow)
                        else:
                            for k in range(KO // 2):
                                nc.tensor.matmul(
                                    ps[:, :cw],
                                    wsb[:, o, 2 * k:2 * k + 2],
                                    xtile[:, 2 * k:2 * k + 2, :cw],
                                    start=(k == 0),
                                    stop=(k == KO // 2 - 1),
                                    perf_mode=mybir.MatmulPerfMode.DoubleRow)
                        nc.vector.tensor_scalar(
                            ot[:, o, :cw],
                            ps[:, :cw],
                            bsb[:, o:o + 1],
                            0.0,
                            mybir.AluOpType.add,
                            mybir.AluOpType.max)
                    nc.sync.dma_start(yt[ci][:, :, :], ot[:, :, :cw])
    nc.compile()
    return nc


def _get_kernel(capsb, capsf):
    key = (capsb, capsf)
    if key not in _KERNEL_CACHE:
        _KERNEL_CACHE[key] = _build(capsb, capsf)
    return _KERNEL_CACHE[key]


def _deal(costs, pad):
    # 32 units (8 experts x 4 quarters) -> 4 slots of 8, sorted by cost.
    # Slots are then reversed so slot 0 is the SMALLEST (head fill) and
    # the last slot the largest.
    units = sorted(((costs[e], e, q) for e in range(E) for q in range(NSLOTS)),
                   key=lambda t: (-t[0], t[1], t[2]))
    slots = [units[8 * s:8 * s + 8] for s in range(NSLOTS)]
    caps = [max(pad, ((sl[0][0] + pad - 1) // pad) * pad) for sl in slots]
    slots.reverse()
    caps.reverse()
    return slots, tuple(caps)


def kernel(x, W_experts, b_experts, W_gate, b_gate):
    global LAST_RESULTS
    x = np.asarray(x, dtype=np.float32)
    W_experts = np.asarray(W_experts, dtype=np.float32)
    b_experts = np.asarray(b_experts, dtype=np.float32)

    vals, idx = _routing(x, W_gate, b_gate)

    wsort = np.sort(vals.ravel().astype(np.float64))
    cum = np.cumsum(wsort**2) / (wsort**2).sum()
    fcap = (TARGET_ERR**2 - BASE_ERR**2) / K_ERR
    i = int(np.searchsorted(cum, fcap))
    tw = float(wsort[min(i, len(wsort) - 1)])

    rows_c = {"b": [], "f": []}
    wvals_c = {"b": [], "f": []}
    counts_c = {"b": [], "f": []}
    for e in range(E):
        mask = idx == e                       # [N, K]
        tok = mask.any(axis=1)
        rows = np.nonzero(tok)[0]
        w = vals[mask]                        # aligned with rows
        lo = w < tw
        rows_c["f"].append(rows[lo]); wvals_c["f"].append(w[lo])
        rows_c["b"].append(rows[~lo]); wvals_c["b"].append(w[~lo])
        counts_c["f"].append(int(lo.sum()))
        counts_c["b"].append(int((~lo).sum()))

    # Per-expert class-boundary optimization: the deal pads each slot to
    # the larger expert of its sorted pair, in BOTH classes.  Shifting a
    # few boundary tokens (w ~ tw) between classes per expert can zero
    # most of that padding.  Hill-climb on the per-expert shift vector,
    # minimizing per-core MM cycles subject to a hard predicted-error cap
    # (the same w^2-mass model used for the threshold itself).
    w2tot = float((vals.astype(np.float64) ** 2).sum())
    base_mass = sum(float((w.astype(np.float64) ** 2).sum())
                    for w in wvals_c["f"])
    # prefix sums of w^2 for move candidates, per expert:
    #  b->f moves take the LOWEST-w bf16 tokens; f->b the HIGHEST-w fp8.
    bw_sorted = [np.sort(wvals_c["b"][e].astype(np.float64)) for e in range(E)]
    fw_sorted = [np.sort(wvals_c["f"][e].astype(np.float64))[::-1]
                 for e in range(E)]
    bpre = [np.concatenate([[0.0], np.cumsum(w ** 2)]) for w in bw_sorted]
    fpre = [np.concatenate([[0.0], np.cumsum(w ** 2)]) for w in fw_sorted]
    ERR_CAP = 0.019

    def mm_cost(cb, cf):
        sb = sorted(cb, reverse=True)
        sf = sorted(cf, reverse=True)
        c = 0
        for k in range(E // 2):
            c += 64 * max(sb[2 * k], sb[2 * k + 1])
            c += 32 * max(sf[2 * k], sf[2 * k + 1])
        return c

    def err_pred(delta):
        m = base_mass
        for e in range(E):
            d = delta[e]
            if d > 0:
                m += bpre[e][d]
            elif d < 0:
                m -= fpre[e][-d]
        return (K_ERR * m / w2tot + BASE_ERR ** 2) ** 0.5

    import random
    rng_ = random.Random(0)
    delta = [0] * E
    cb = [counts_c["b"][e] for e in range(E)]
    cf = [counts_c["f"][e] for e in range(E)]
    cur = mm_cost(cb, cf)
    best_delta, best_cost = delta[:], cur
    for _ in range(6000):
        e = rng_.randrange(E)
        st = rng_.choice((-48, -16, -4, 4, 16, 48))
        nd = delta[:]
        nd[e] += st
        if nd[e] > len(bw_sorted[e]) or -nd[e] > len(fw_sorted[e]):
            continue
        if err_pred(nd) > ERR_CAP:
            continue
        nb = [counts_c["b"][i] - nd[i] for i in range(E)]
        nf = [counts_c["f"][i] + nd[i] for i in range(E)]
        nc_ = mm_cost(nb, nf)
        if nc_ <= cur:
            delta, cur = nd, nc_
            if nc_ < best_cost:
                best_delta, best_cost = nd[:], nc_
    for e in range(E):
        d = best_delta[e]
        if d > 0:    # move d lowest-w bf16 tokens -> fp8
            w = wvals_c["b"][e]
            mv = np.argsort(w)[:d]
            keep = np.ones(len(w), dtype=bool)
            keep[mv] = False
            rows_c["f"][e] = np.concatenate([rows_c["f"][e], rows_c["b"][e][mv]])
            wvals_c["f"][e] = np.concatenate([wvals_c["f"][e], w[mv]])
            rows_c["b"][e] = rows_c["b"][e][keep]
            wvals_c["b"][e] = w[keep]
        elif d < 0:  # move |d| highest-w fp8 tokens -> bf16
            w = wvals_c["f"][e]
            mv = np.argsort(w)[len(w) + d:]
            keep = np.ones(len(w), dtype=bool)
            keep[mv] = False
            rows_c["b"][e] = np.concatenate([rows_c["b"][e], rows_c["f"][e][mv]])
            wvals_c["b"][e] = np.concatenate([wvals_c["b"][e], w[mv]])
            rows_c["f"][e] = rows_c["f"][e][keep]
            wvals_c["f"][e] = w[keep]
        counts_c["b"][e] -= d
        counts_c["f"][e] += d

    slotsb, capsb = _deal(counts_c["b"], 4)
    slotsf, capsf = _deal(counts_c["f"], 16)
    nc = _get_kernel(capsb, capsf)

    ctlb, ctlf = _ctl_lists(capsb, capsf)

    xbf = np.ascontiguousarray(x.astype(BF16))
    xf8 = np.ascontiguousarray(np.clip(x * SX, -240, 240).astype(F8))
    W8 = np.clip(W_experts * SW, -240, 240).astype(F8)
    Wb = W_experts.astype(BF16)

    def pack_x(src, rows):
        cnt = len(rows)
        return np.ascontiguousarray(
            src[rows].T.reshape(KO, P, cnt).transpose(1, 0, 2))

    xpack = {}
    for cls in ("b", "f"):
        src = xbf if cls == "b" else xf8
        for e in range(E):
            xpack[(cls, e)] = pack_x(src, rows_c[cls][e])

    in_maps = [{} for _ in range(8)]
    for cls, slots, caps, ctl in (("b", slotsb, capsb, ctlb),
                                  ("f", slotsf, capsf, ctlf)):
        Wsrc = Wb if cls == "b" else W8
        dt = BF16 if cls == "b" else F8
        bscale = 1.0 if cls == "b" else SCOMB
        for s in range(NSLOTS):
            C = caps[s]
            for i in range(8):
                cnt, e, q = slots[s][i]
                xe = np.zeros((P, KO, C), dtype=dt)
                xe[:, :, :cnt] = xpack[(cls, e)]
                arr = Wsrc[e][q * QW:(q + 1) * QW].reshape(OTS, P, KO, P)
                if cls == "f" and s == 0:
                    we = np.ascontiguousarray(
                        arr.transpose(2, 3, 0, 1)
                        .reshape(4, 4, P, OTS, P)
                        .transpose(0, 2, 3, 1, 4))
                else:
                    we = np.ascontiguousarray(arr.transpose(0, 3, 2, 1))
                be = np.ascontiguousarray(
                    (bscale * b_experts[e][q * QW:(q + 1) * QW])
                    .astype(np.float32).reshape(OTS, P).T)
                pre = "wtb" if cls == "b" else "wtf"
                xpre = "xtb" if cls == "b" else "xtf"
                bpre = "bb" if cls == "b" else "bf"
                in_maps[i][f"{pre}{s}"] = we
                in_maps[i][f"{bpre}{s}"] = be
                for ci, (c0, cw) in enumerate(ctl[s]):
                    in_maps[i][f"{xpre}{s}_{ci}"] = np.ascontiguousarray(
                        xe[:, :, c0:c0 + cw])

    from concourse.bass_utils import run_bass_kernel_spmd

    res = run_bass_kernel_spmd(nc, in_maps, core_ids=list(range(8)), trace=PROFILE)
    LAST_RESULTS = res

    out = np.zeros((N, D), dtype=np.float32)
    for cls, slots, ctl in (("b", slotsb, ctlb), ("f", slotsf, ctlf)):
        ypre = "ytb" if cls == "b" else "ytf"
        wdiv = 1.0 if cls == "b" else SCOMB
        for s in range(NSLOTS):
            for i in range(8):
                cnt, e, q = slots[s][i]
                if cnt == 0:
                    continue
                yt_si = np.concatenate(
                    [res.results[i][f"{ypre}{s}_{ci}"]
                     for ci in range(len(ctl[s]))], axis=2)  # [P, OTS, C]
                y = (yt_si[:, :, :cnt].astype(np.float32)
                     .transpose(2, 1, 0).reshape(cnt, QW))
                # rows for this (cls, e); each unit of expert e uses the
                # same token set across its 4 quarters
                r = rows_c[cls][e]
                w = wvals_c[cls][e].astype(np.float32) / wdiv
                out[r, q * QW:(q + 1) * QW] += w[:, None] * y
    return out
